# revision 24
# baseline (speedup 1.0000x reference)
"""Trainium2 Bass kernel for nn_ARNet (EGNN coordinate-update layer).

Math reduction
--------------
reference: coors = [x, x] (dup 3D -> 6D); per edge (i, j):
  rel_dist d_ij = ||coors_i - coors_j||^2 = 2*||x_i - x_j||^2
  w = clip(MLP(d), -2, 2)        (scalar->scalar MLP: 1->32->32->gate->128->1)
  out_i = coors_i + sum_j w * pm_ij * (coors_i - coors_j) / max(sqrt(d), 1e-8) * scale
  return out * mask

Everything per-edge is a scalar function of d.  With this problem's weights
(all biases zero => f(0)=0) f crosses |f|=2 once and is monotone beyond, so
clip(f(d)) == clip(p(min(d, DCAP)), +-2) where p(u) = u*q(u) is a degree-9
polynomial fit on [0, DCAP] (validated at runtime against the actual weights,
residual ~1e-4).  Defining G_ij = clip*scale/max(sqrt(d),eps) (symmetric!),
  out_i = m_i*x_i*(1 + S_i) - m_i*A_i,  [A|S] = G @ [m*x, m].

Device pipeline per 128x512 tile of the [N, N] pairwise block (2 molecules):
  PE : D = WL^T @ WR        (Gram trick, K=5: [x, |x|^2, 1] x [-4x, 2, 2|x|^2])
  ACT: dp = relu(D);  s = sqrt(dp + 1e-16)
  DVE: rc = 1/s (fast approx custom op);  t = min(dp, DCAP)*(1/DCAP)
       scalar_tensor_tensor chain: z = (z + c_k) * t   (Horner for u*q(u))
       G = min(z, 2*scale) * rc
  PE : [A | S] = G @ [m*x, m]  (PSUM-accumulated over j-blocks; G symmetric
       so its [i, j] tiles serve directly as lhsT)
  DVE: res = m*x*(1+S) - m*A
Output [i, 0:3] = [i, 3:6] = res.

Sharding: data-parallel over batch, 8 molecules per core on 8 cores; the tiny
O(B*N) per-node feature rows ([x, |x|^2, 1, -4x, 2, 2|x|^2] and [m*x, m]) are
prepared host-side so each matmul operand tile has exactly one DMA producer
(the PE LDWEIGHTS instruction supports a single sync-wait).
"""

import numpy as np

import concourse.bass as bass
import concourse.bacc as bacc
import concourse.mybir as mybir
import concourse.tile as tile
from concourse.bass_utils import run_bass_kernel_spmd

F32 = mybir.dt.float32
ALU = mybir.AluOpType
ACTF = mybir.ActivationFunctionType

B, N, NCORES = 64, 256, 8
BPC = B // NCORES          # molecules per core
PAIRS = BPC // 2           # molecule pairs per core (2 molecules per hot tile)
L = BPC * N                # nodes per core
DEG = 8                    # degree of q in p(u) = u * q(u)
EPS2 = 1e-16               # folded into sqrt: s = sqrt(relu(D) + EPS2)


def _mlp_f64(weights):
    W1 = np.asarray(weights["W1"], np.float64); b1 = np.asarray(weights["b1"], np.float64)
    W2 = np.asarray(weights["W2"], np.float64); b2 = np.asarray(weights["b2"], np.float64)
    Wg = np.asarray(weights["Wg"], np.float64); bg = np.asarray(weights["bg"], np.float64)
    Wc1 = np.asarray(weights["Wc1"], np.float64); bc1 = np.asarray(weights["bc1"], np.float64)
    Wc2 = np.asarray(weights["Wc2"], np.float64); bc2 = np.asarray(weights["bc2"], np.float64)

    def lips(v):
        return 0.909 * v / (1 + np.exp(-v))

    def f(dv):
        dv = np.asarray(dv, np.float64)[:, None]
        m = lips(dv @ W1 + b1)
        m = lips(m @ W2 + b2)
        m = m / (1 + np.exp(-(m @ Wg + bg)))
        h = lips(m @ Wc1 + bc1)
        return (h @ Wc2 + bc2)[:, 0]

    return f


def _fit_chain(weights, scale, dmax):
    """Fit f on [0, DCAP]; return (chain coeffs c_1..c_{DEG+1}, HI, DCAP).

    Chain: z = c_1*t; z = (z + c_k)*t ... => z = sum_k c_k t^(M+2-k), t=u/DCAP.
    Valid when f(0) == 0 (true here: zero biases) and clip(f) is constant
    beyond DCAP (validated below)."""
    f = _mlp_f64(weights)
    grid = np.linspace(0, max(dmax, 8.0), 8192)
    fg = f(grid)
    thresh = np.abs(fg) >= 2.05
    dcap = None
    for v in np.arange(4.0, max(dmax, 8.0) + 1, 0.5):
        sel = grid >= v - 0.25
        if thresh[sel].all():
            dcap = float(v)
            break
    if dcap is None:
        dcap = float(max(dmax, 8.0))  # fit whole range (degraded accuracy)
    n = 4000
    k = np.arange(n)
    u = dcap / 2 * (1 - np.cos(np.pi * (k + 0.5) / n))
    u = np.clip(u, 1e-9, None)
    g = f(u) / u
    cheb = np.polynomial.chebyshev.Chebyshev.fit(u, g, DEG, domain=[0, dcap])
    a = cheb.convert(kind=np.polynomial.Polynomial).coef
    if len(a) < DEG + 1:
        a = np.concatenate([a, np.zeros(DEG + 1 - len(a))])
    j = np.arange(1, DEG + 2)
    b_coef = a * (dcap ** j) * scale          # p(t) = sum_j b_j t^j (scale folded)
    c = b_coef[::-1].copy()                   # c_k = b_{M+2-k}
    # validate fit in fp32 chain semantics
    ut = np.linspace(0, dcap, 20001)
    t32 = (np.minimum(ut, dcap) / dcap).astype(np.float32)
    z = (t32 * np.float32(c[0])).astype(np.float32)
    for ck in c[1:]:
        z = ((z + np.float32(ck)) * t32).astype(np.float32)
    err = np.abs(z - f(ut) * scale)
    act = np.abs(f(ut)) <= 2.05
    assert err[act].max() < 5e-3, f"poly fit too inaccurate: {err[act].max()}"
    sat_ok = (z[~act] >= 2.0 * scale).all() if scale > 0 else (z[~act] <= 2.0 * scale).all()
    assert sat_ok, "fit does not stay saturated beyond clip crossing"
    return [float(v) for v in c], float(2.0 * abs(scale)), dcap


def _build_bass(c, hi, dcap, pos_scale, loop_iters=None):
    nc = bacc.Bacc(None, target_bir_lowering=False)
    # register sqrt-bias epsilon as a preamble const AP (no runtime producer,
    # so the Sqrt activation needs no extra sync wait for its bias operand)
    eps_tensor = nc.alloc_sbuf_tensor("const-eps", [128, 1], F32)
    nc.gpsimd.memset(eps_tensor.ap(), EPS2)
    nc.const_aps.aps[(F32, EPS2)] = eps_tensor.ap()
    wf = nc.dram_tensor("wf", [5, 2 * L], F32, kind="ExternalInput")
    xam = nc.dram_tensor("xam", [L, 4], F32, kind="ExternalInput")
    out = nc.dram_tensor("o", [L, 6], F32, kind="ExternalOutput")
    clip_op = ALU.min if pos_scale else ALU.max
    clip_lim = hi if pos_scale else -hi

    with tile.TileContext(nc) as tc:
        with (
            tc.tile_pool(name="singles", bufs=1) as singles,
            tc.tile_pool(name="mlp", bufs=3) as mpool,
            tc.tile_pool(name="gt", bufs=5) as gpool,
            tc.tile_pool(name="fin", bufs=6) as rpool,
            tc.tile_pool(name="dps", bufs=4, space="PSUM") as dpool,
            tc.tile_pool(name="aps", bufs=4, space="PSUM") as apool,
        ):
            WW = singles.tile([5, 2 * L], F32)   # [WL | WR] feature rows
            nc.sync.dma_start(out=WW[:], in_=wf[:])
            xad = singles.tile([128, BPC * 2 * 4], F32)
            nc.sync.dma_start(
                out=xad[:],
                in_=bass.AP(tensor=xam[:].tensor, offset=0,
                            ap=[[4, 128], [512, BPC * 2], [1, 4]]))
            xat = singles.tile([128, BPC * 2 * 4], F32)
            nc.vector.tensor_copy(xat[:], xad[:])   # single-producer (DVE)

            import contextlib
            loop_cm = (tc.For_i(0, loop_iters, 1) if loop_iters
                       else contextlib.nullcontext())
            with loop_cm:
                _emit_body(nc, tc, mpool, gpool, rpool, dpool, apool,
                           WW, xat, out, c, hi, dcap, pos_scale)
    nc.finalize()
    return nc


def _emit_body(nc, tc, mpool, gpool, rpool, dpool, apool,
               WW, xat, out, c, hi, dcap, pos_scale):
    clip_op = ALU.min if pos_scale else ALU.max
    clip_lim = hi if pos_scale else -hi
    if True:
        if True:
            for P in range(PAIRS):
                mols = (2 * P, 2 * P + 1)
                gtiles = []
                for q in range(2):
                    Dps = dpool.tile([128, 512], F32)
                    for h, bb in enumerate(mols):
                        o = bb * N + q * 128
                        nc.tensor.matmul(
                            Dps[:, h * 256:(h + 1) * 256],
                            WW[:, o:o + 128],
                            WW[:, L + bb * N: L + bb * N + 256],
                            start=True, stop=True)
                    dp = mpool.tile([128, 512], F32)
                    nc.scalar.activation(dp[:], Dps[:], ACTF.Relu)
                    s = mpool.tile([128, 512], F32)
                    nc.scalar.activation(s[:], dp[:], ACTF.Sqrt, bias=EPS2)
                    rc = mpool.tile([128, 512], F32)
                    nc.vector.reciprocal_approx_fast(rc[:], s[:])
                    t = mpool.tile([128, 512], F32)
                    nc.vector.tensor_scalar(
                        t[:], dp[:], dcap, 1.0 / dcap, op0=ALU.min, op1=ALU.mult)
                    z = mpool.tile([128, 512], F32)
                    nc.vector.tensor_scalar_mul(z[:], t[:], c[0])
                    for ck in c[1:]:
                        nc.vector.scalar_tensor_tensor(
                            z[:], z[:], ck, t[:], op0=ALU.add, op1=ALU.mult)
                    g = gpool.tile([128, 512], F32)
                    nc.vector.scalar_tensor_tensor(
                        g[:], z[:], clip_lim, rc[:], op0=clip_op, op1=ALU.mult)
                    gtiles.append(g)

                # aggregation: [A|S] = G @ [m*x, m];  res = m*x*(1+S) - m*A
                for h, bb in enumerate(mols):
                    for qo in range(2):
                        aps = apool.tile([128, 4], F32)
                        for r in range(2):
                            o = h * 256 + qo * 128
                            k4 = (bb * 2 + r) * 4
                            nc.tensor.matmul(
                                aps[:],
                                gtiles[r][:, o:o + 128],
                                xat[:, k4:k4 + 4],
                                start=(r == 0), stop=(r == 1))
                        k4 = (bb * 2 + qo) * 4
                        xsl = xat[:, k4:k4 + 3]
                        msl = xat[:, k4 + 3:k4 + 4]
                        r1 = rpool.tile([128, 3], F32)
                        nc.vector.scalar_tensor_tensor(
                            r1[:], xsl, aps[:, 3:4], xsl, op0=ALU.mult, op1=ALU.add)
                        am = rpool.tile([128, 3], F32)
                        nc.vector.tensor_scalar_mul(am[:], aps[:, 0:3], msl)
                        res = rpool.tile([128, 3], F32)
                        nc.vector.tensor_sub(res[:], r1[:], am[:])
                        rows = bb * N + qo * 128
                        nc.sync.dma_start(out=out[rows:rows + 128, 0:3], in_=res[:])
                        nc.sync.dma_start(out=out[rows:rows + 128, 3:6], in_=res[:])


def _prepare(inputs, loop_iters=None):
    x = np.ascontiguousarray(np.asarray(inputs["x"], np.float32))
    mask = np.ascontiguousarray(np.asarray(inputs["mask"], np.float32))
    scale = float(np.asarray(inputs["scale"]))
    assert scale != 0.0
    r = np.sqrt((x * x).sum(-1)).max()
    dmax = float(2.0 * (2.0 * r) ** 2)     # bound on max pairwise rel_dist
    c, hi, dcap = _fit_chain(inputs, scale, dmax)
    nc = _build_bass(c, hi, dcap, pos_scale=scale > 0, loop_iters=loop_iters)

    xf = x.reshape(B * N, 3)
    nsq = (xf * xf).sum(-1)
    onesc = np.ones_like(nsq)
    wl = np.stack([xf[:, 0], xf[:, 1], xf[:, 2], nsq, onesc])          # [5, B*N]
    wr = np.stack([-4 * xf[:, 0], -4 * xf[:, 1], -4 * xf[:, 2],
                   2 * onesc, 2 * nsq])                                # [5, B*N]
    mf = mask.reshape(B * N)
    xam_full = np.concatenate([xf * mf[:, None], mf[:, None]], 1)      # [B*N, 4]

    in_maps = []
    for core in range(NCORES):
        sl = slice(core * L, (core + 1) * L)
        wf = np.concatenate([wl[:, sl], wr[:, sl]], 1)                 # [5, 2L]
        in_maps.append({
            "wf": np.ascontiguousarray(wf, np.float32),
            "xam": np.ascontiguousarray(xam_full[sl], np.float32),
        })
    return nc, in_maps


def _run(inputs, trace=False, **kw):
    nc, in_maps = _prepare(inputs)
    res = run_bass_kernel_spmd(nc, in_maps, core_ids=list(range(NCORES)),
                               trace=trace, **kw)
    out = np.concatenate(
        [r["o"].reshape(BPC, N, 6) for r in res.results], axis=0)
    return out, res


def kernel(**inputs) -> np.ndarray:
    out, _ = _run(inputs)
    return out
